# revision 26
# baseline (speedup 1.0000x reference)
"""MultiHeadedAttention Trainium2 kernel (v2: slot-pipelined).

Problem: B=2, T=2048, D=1024, H=16 heads (DK=64), fp32 in/out, padding mask
on keys. out = softmax(mask(QWq (KWk)^T / 8)) @ (VWv) @ Wo^T + biases.

Sharding (8 cores): core c -> batch b = c//4, head group g = c%4 (4 heads,
256 projection columns). Each core computes its heads' attention and a
partial output projection; host sums the 4 partials per batch (+ bo).

v2 design (vs baseline):
  - Flat 128-slot pipeline: 8 units (4 query-blocks of 512 x 2 head-pairs)
    x 16 key-chunks.  Per slot: 2 concurrent score matmuls (row-tiled at
    partitions 0/64, K=64 each), ONE exp activation N=1024 covering both
    heads, and ~2 lagged attn@V matmul pairs.  ScalarE (exp) is the pacer
    at ~1.15us/slot; everything else hides underneath.
  - PSUM budget = exactly 8 banks: score ping-pong 2x[128,2,512]f32 (4),
    o2 accumulator [65,2,512]f32 (2), borrow slot [128,1024]f32 (2).
  - Projections (beyond a minimal startup prefix), v-projection and the
    output projection are emitted as "borrows" of the borrow slot, woven
    between slots at statically scheduled points.
  - Startup prefix: kT(m0) + qT(m0, qblock0) only -> first exp at ~17us
    (baseline: 36us).
  - attn@V lags exp by ~3 slots in steady state (elastic early on while
    xv/wv DMAs land), so the tail after the last exp is tiny.
  - Output DMA'd as bf16 (host sums partials in fp32): halves out traffic.
"""

import numpy as np
import ml_dtypes

import concourse.bass as bass
import concourse.bacc as bacc
import concourse.tile as tile
from concourse import mybir
from concourse.bass_utils import run_bass_kernel_spmd

B, T, D, H = 2, 2048, 1024, 16
DK = D // H  # 64
GH = 4       # heads per core
GC = GH * DK  # 256 proj columns per core
NCORES = 8
KC = T // 128   # 16 key chunks
DCH = D // 128  # 8 contraction chunks
NQB = 4         # query blocks of 512
NU = 2 * NQB    # units: u = 2*qb + pr
NSLOT = NU * KC
F32 = mybir.dt.float32
BF16 = mybir.dt.bfloat16

MASK_NEG = -30000.0


def build_program(with_bv: bool, debug_taps: bool = False):
    nc = bacc.Bacc("TRN2")

    # ---- DRAM parameters (per-core shapes) ----
    xq_d = nc.declare_dram_parameter("xq", [NQB, DCH, 128, 512], BF16,
                                     isOutput=False)
    xk_d = nc.declare_dram_parameter("xk", [DCH, 128, T], BF16, isOutput=False)
    xv_d = nc.declare_dram_parameter("xv", [NQB, DCH, 128, 512], BF16,
                                     isOutput=False)
    wq_d = nc.declare_dram_parameter("wq", [128, 2, DCH, 128], BF16,
                                     isOutput=False)
    wk_d = nc.declare_dram_parameter("wk", [128, 2, DCH, 128], BF16,
                                     isOutput=False)
    wv_d = nc.declare_dram_parameter("wv", [128, DCH, GC], BF16, isOutput=False)
    wo_d = nc.declare_dram_parameter("wo", [64, 2, 2, D], BF16,
                                     isOutput=False)
    mask_d = nc.declare_dram_parameter("maskb", [128, KC], F32, isOutput=False)
    bq_d = nc.declare_dram_parameter("bq", [128, 2], F32, isOutput=False)
    bk_d = nc.declare_dram_parameter("bk", [128, 2], F32, isOutput=False)
    bv_d = nc.declare_dram_parameter("bv", [64, GH], F32, isOutput=False)
    # transposed [D, T]: the host transposes back.  This keeps xh as the
    # MOVING operand of the output projection (wo is stationary), which
    # keeps the xh consumer robustly ordered after the norm writes.
    out_d = nc.declare_dram_parameter("out", [D, T], BF16, isOutput=True)
    if debug_taps:
        tap_qT = nc.declare_dram_parameter("tap_qT", [128, 2, T], BF16,
                                           isOutput=True)
        tap_kT = nc.declare_dram_parameter("tap_kT", [128, 2, T], BF16,
                                           isOutput=True)
        tap_v = nc.declare_dram_parameter("tap_v", [128, KC, GH, 66], BF16,
                                          isOutput=True)
        tap_xh = [nc.declare_dram_parameter(f"tap_xh{q}", [64, 2, 2, 1024],
                                            BF16, isOutput=True)
                  for q in (0, 1)]

    EXPF = mybir.ActivationFunctionType.Exp

    with tile.TileContext(nc) as tc:
        with (
            tc.tile_pool(name="persist", bufs=1) as pp,
            tc.tile_pool(name="psum", bufs=1, space="PSUM") as psp,
            tc.tile_pool(name="expool", bufs=17) as exp_pool,
            tc.tile_pool(name="normp", bufs=1) as norm_pool,
            tc.tile_pool(name="outp", bufs=2) as out_pool,
        ):
            # persistent sbuf tensors
            wq_sb = pp.tile([128, 2, DCH, 128], BF16, tag="wq")
            wk_sb = pp.tile([128, 2, DCH, 128], BF16, tag="wk")
            wv_sb = pp.tile([128, DCH, GC], BF16, tag="wv")
            wo_sb = pp.tile([64, 2, 2, D], BF16, tag="wo")
            mask_sb = pp.tile([128, KC], F32, tag="mask")
            bq_sb = pp.tile([128, 2], F32, tag="bq")
            bk_sb = pp.tile([128, 2], F32, tag="bk")
            bv_sb = pp.tile([64, GH], F32, tag="bv")
            qT_sb = pp.tile([128, 2, T], BF16, tag="qT")
            kT_sb = pp.tile([128, 2, T], BF16, tag="kT")
            v_sb = pp.tile([128, KC, GH, 66], BF16, tag="v")
            # all heads' normalized outputs live at partitions 0:64
            # ([hh, pr] in the free dims) so every write is a plain DVE op
            # and the out-projection contracts K=64 per (hh, m) pair.
            xh_sb = [pp.tile([64, 2, 2, 1024], BF16, tag=f"xh{q}",
                             name=f"xh{q}") for q in (0, 1)]
            xk_sb = [pp.tile([128, T], BF16, tag=f"xk{k}", name=f"xk{k}")
                     for k in range(DCH)]
            xq_sb = [pp.tile([128, T], BF16, tag=f"xq{k}", name=f"xq{k}")
                     for k in range(DCH)]
            xv_sb = [pp.tile([128, T], BF16, tag=f"xv{k}", name=f"xv{k}")
                     for k in range(DCH)]
            nc.vector.memset(v_sb[:, :, :, 64:65], 1.0)

            # dummy exp to pull the ACT table load into the DMA-wait window
            dmy = pp.tile([128, 16], F32, tag="dmy")
            dmy2 = pp.tile([128, 16], BF16, tag="dmy2")
            nc.vector.memset(dmy[:], 0.0)
            nc.scalar.activation(dmy2[:], dmy[:], EXPF)

            # ---- DMA schedule (issue order matters: queue is FIFO) ----
            nc.sync.dma_start(out=wk_sb[:, 0], in_=wk_d[:, 0])
            nc.sync.dma_start(out=wq_sb[:, 0], in_=wq_d[:, 0])
            for k in range(DCH):
                nc.sync.dma_start(out=xk_sb[k][:], in_=xk_d[k])
            for k in range(DCH):  # q block 0
                nc.sync.dma_start(out=xq_sb[k][:, 0:512], in_=xq_d[0, k])
            nc.sync.dma_start(out=mask_sb[:], in_=mask_d[:])
            nc.sync.dma_start(out=bk_sb[:], in_=bk_d[:])
            nc.sync.dma_start(out=bq_sb[:], in_=bq_d[:])
            nc.sync.dma_start(out=wk_sb[:, 1], in_=wk_d[:, 1])
            nc.sync.dma_start(out=wq_sb[:, 1], in_=wq_d[:, 1])
            nc.sync.dma_start(out=wv_sb[:], in_=wv_d[:])
            for k in range(DCH):  # v block 0
                nc.sync.dma_start(out=xv_sb[k][:, 0:512], in_=xv_d[0, k])
            for qb in (1, 2):
                for k in range(DCH):
                    nc.sync.dma_start(out=xq_sb[k][:, qb * 512:(qb + 1) * 512],
                                      in_=xq_d[qb, k])
                for k in range(DCH):
                    nc.sync.dma_start(out=xv_sb[k][:, qb * 512:(qb + 1) * 512],
                                      in_=xv_d[qb, k])
            for k in range(DCH):
                nc.sync.dma_start(out=xq_sb[k][:, 3 * 512:4 * 512],
                                  in_=xq_d[3, k])
            nc.sync.dma_start(out=wo_sb[:], in_=wo_d[:])
            for k in range(DCH):
                nc.sync.dma_start(out=xv_sb[k][:, 3 * 512:4 * 512],
                                  in_=xv_d[3, k])
            nc.sync.dma_start(out=bv_sb[:], in_=bv_d[:])

            # ---- helpers ----
            def emit_proj(dst, m, w_sb, x_sb, c0, width, b_sb, tag):
                """dst = (W_m^T x)[:, c0:c0+width] + bias_m."""
                pst = psp.tile([128, 1024], F32, tag=tag,
                               bufs=1 if tag in ("br", "o2") else 2,
                               name="pst")
                nhalf = width // 512
                for k in range(DCH):
                    for n in range(nhalf):
                        nc.tensor.matmul(
                            pst[:, n * 512:(n + 1) * 512],
                            w_sb[:, m, k, :],
                            x_sb[k][:, c0 + n * 512:c0 + (n + 1) * 512],
                            start=(k == 0), stop=(k == DCH - 1),
                            skip_group_check=True,
                        )
                nc.vector.tensor_scalar_add(
                    dst[:], pst[:, 0:width], b_sb[:, m:m + 1])

            def emit_vproj(g):
                """v projection for token chunks 4g..4g+3 (= key chunks)."""
                vps = psp.tile([128, 4, GH, 64], F32, tag="br", bufs=1,
                               name="vps")
                for t in range(4):
                    tcn = 4 * g + t
                    for k in range(DCH):
                        nc.tensor.matmul(
                            vps[:, t, :, :],
                            xv_sb[k][:, tcn * 128:(tcn + 1) * 128],
                            wv_sb[:, k, :],
                            start=(k == 0), stop=(k == DCH - 1),
                            skip_group_check=True,
                        )
                nc.vector.tensor_copy(v_sb[:, 4 * g:4 * g + 4, :, 0:64],
                                      vps[:])

            def emit_outproj(job, tail=False):
                """partial out^T for token block qb, d-chunks 2*dcg..2*dcg+1.

                po[dout, t] = sum_m sum_p wo[p, m, dout] * xh[p, m, t]:
                wo is the stationary operand, xh the moving one."""
                qb, dcg = job // 4, job % 4
                qh, off = qb // 2, (qb % 2) * 512
                po = psp.tile([128, 2, 512], F32, tag="br", bufs=1, name="po")
                for d2 in range(2):
                    dc = 2 * dcg + d2
                    for i, (m, hh) in enumerate(
                            ((0, 0), (0, 1), (1, 0), (1, 1))):
                        nc.tensor.matmul(
                            po[:, d2, :],
                            wo_sb[0:64, hh, m, dc * 128:(dc + 1) * 128],
                            xh_sb[qh][0:64, hh, m, off:off + 512],
                            start=(i == 0), stop=(i == 3),
                            skip_group_check=True,
                        )
                ot = out_pool.tile([128, 2, 512], BF16, tag="ot")
                if tail and job % 2 == 0:
                    nc.scalar.copy(ot[:], po[:])
                else:
                    nc.vector.tensor_copy(ot[:], po[:])
                for d2 in range(2):
                    dc = 2 * dcg + d2
                    nc.sync.dma_start(
                        out=out_d[dc * 128:(dc + 1) * 128,
                                  qb * 512:(qb + 1) * 512],
                        in_=ot[:, d2, :])

            def emit_norm(u, o2):
                """normalize o2 -> xh for unit u=(qb,pr)."""
                qb, pr = u // 2, u % 2
                qh, off = qb // 2, (qb % 2) * 512
                rd = norm_pool.tile([1, 2, 512], F32, tag="rd", name="rd")
                rc = norm_pool.tile([1, 2, 512], F32, tag="rc", name="rc")
                nc.vector.tensor_copy(rd[:], o2[64:65, :, :])
                nc.vector.reciprocal_approx_fast(rc[:], rd[:])
                rb = norm_pool.tile([64, 2, 512], F32, tag="rb", name="rb")
                nc.gpsimd.partition_broadcast(rb[:], rc[:])
                for hh in range(2):
                    dst = xh_sb[qh][0:64, hh, pr, off:off + 512]
                    nc.vector.tensor_mul(dst, o2[0:64, hh, :], rb[:, hh, :])
                    if with_bv:
                        nc.vector.tensor_scalar_add(
                            dst, dst, bv_sb[:, 2 * pr + hh:2 * pr + hh + 1])

            # ---- startup prefix: kT(m0) full + qT(m0, qb0) ----
            # the two 1024-key halves are interleaved chunk-by-chunk so the
            # matmuls trail the xk DMA stream with no serialization.
            pst0 = psp.tile([128, 1024], F32, tag="br", bufs=1, name="pst0")
            pst1 = psp.tile([128, 1024], F32, tag="sc", bufs=2, name="pst1")
            for k in range(DCH):
                for half, pst in ((0, pst0), (1, pst1)):
                    for n in range(2):
                        c = half * 1024 + n * 512
                        nc.tensor.matmul(
                            pst[:, n * 512:(n + 1) * 512],
                            wk_sb[:, 0, k, :], xk_sb[k][:, c:c + 512],
                            start=(k == 0), stop=(k == DCH - 1),
                            skip_group_check=True,
                        )
            nc.vector.tensor_scalar_add(kT_sb[:, 0, 0:1024], pst0[:],
                                        bk_sb[:, 0:1])
            nc.vector.tensor_scalar_add(kT_sb[:, 0, 1024:2048], pst1[:],
                                        bk_sb[:, 0:1])
            emit_proj(qT_sb[:, 0, 0:512], 0, wq_sb, xq_sb, 0, 512, bq_sb,
                      "sc")

            # ---- borrow plan: slot index -> list of thunks ----
            plan = {}

            def at(s, fn, *a, **kw):
                plan.setdefault(s, []).append(lambda: fn(*a, **kw))

            at(3, emit_proj, qT_sb[:, 1, 0:512], 1, wq_sb, xq_sb, 0, 512,
               bq_sb, "br")
            at(7, emit_proj, kT_sb[:, 1, 0:1024], 1, wk_sb, xk_sb, 0, 1024,
               bk_sb, "br")
            at(11, emit_proj, kT_sb[:, 1, 1024:2048], 1, wk_sb, xk_sb, 1024,
               1024, bk_sb, "br")
            at(15, emit_vproj, 0)
            at(19, emit_vproj, 1)
            at(23, emit_proj, qT_sb[:, 0, 512:1024], 0, wq_sb, xq_sb, 512,
               512, bq_sb, "br")
            at(27, emit_vproj, 2)
            at(31, emit_vproj, 3)
            at(35, emit_proj, qT_sb[:, 1, 512:1024], 1, wq_sb, xq_sb, 512,
               512, bq_sb, "br")
            at(39, emit_proj, qT_sb[:, 0, 1024:2048], 0, wq_sb, xq_sb, 1024,
               1024, bq_sb, "br")
            at(43, emit_proj, qT_sb[:, 1, 1024:2048], 1, wq_sb, xq_sb, 1024,
               1024, bq_sb, "br")
            # outproj jobs 0-11 are emitted via a readiness-gated queue (the
            # job must come AFTER both of its units' norms in program order,
            # or Tile sees a read-before-write and drops the dependency).
            # qb3 (jobs 12-15) must wait for norm(u7): tail only.
            out_min_slot = {0: 49, 1: 52, 2: 55, 3: 58,
                            4: 64, 5: 67, 6: 70, 7: 73,
                            8: 102, 9: 105, 10: 108, 11: 111}

            # vproj group g emitted at slot:
            vproj_slot = {0: 15, 1: 19, 2: 27, 3: 31}

            # ---- the slot loop ----
            ex_tiles = {}
            o2_cur = [None]
            vnext = [0]
            VLAG = 3
            norm_slot = {}      # unit -> slot its norm was emitted at
            out_queue = list(range(12))

            def emit_V(t, s):
                u, kc = t // KC, t % KC
                pr = u % 2
                if kc == 0:
                    o2_cur[0] = psp.tile([65, 2, 512], F32, tag="o2", bufs=1,
                                         name="o2")
                o2 = o2_cur[0]
                for hh in range(2):
                    nc.tensor.matmul(
                        o2[:, hh, :],
                        v_sb[:, kc, 2 * pr + hh, 0:65],
                        ex_tiles[t][:, hh, :],
                        start=(kc == 0), stop=(kc == KC - 1),
                        skip_group_check=True,
                    )
                if kc == KC - 1:
                    emit_norm(u, o2)
                    norm_slot[u] = s
                del ex_tiles[t]

            def v_ready(t, s):
                if t > s - VLAG:
                    return False
                if vproj_slot[(t % KC) // 4] + 5 > s:
                    return False
                return True

            for s in range(NSLOT):
                u, kc = s // KC, s % KC
                qb, pr = u // 2, u % 2
                sc = psp.tile([128, 2, 512], F32, tag="sc", bufs=2, name="sc")
                for hh in range(2):
                    nc.tensor.matmul(
                        sc[:, hh, :],
                        kT_sb[64 * hh:64 * hh + 64, pr,
                              kc * 128:(kc + 1) * 128],
                        qT_sb[64 * hh:64 * hh + 64, pr,
                              qb * 512:(qb + 1) * 512],
                        start=True, stop=True,
                    )
                ex = exp_pool.tile([128, 2, 512], BF16, tag="ex", name="ex")
                nc.scalar.activation(ex[:], sc[:], EXPF,
                                     bias=mask_sb[:, kc:kc + 1],
                                     scale=float(DK) ** -0.5)
                ex_tiles[s] = ex
                # lagged attn@V (up to 2 slot-pairs per slot)
                nv = 0
                while vnext[0] < NSLOT and nv < 2 and v_ready(vnext[0], s):
                    emit_V(vnext[0], s)
                    vnext[0] += 1
                    nv += 1
                for fn in plan.get(s, []):
                    fn()
                while out_queue:
                    job = out_queue[0]
                    qb = job // 4
                    us = (2 * qb, 2 * qb + 1)
                    if (s >= out_min_slot[job]
                            and all(norm_slot.get(u, 999) <= s - 2
                                    for u in us)):
                        emit_outproj(out_queue.pop(0))
                    else:
                        break

            # ---- tail: drain V, final norm, qb3 outproj ----
            while vnext[0] < NSLOT:
                emit_V(vnext[0], NSLOT + 8)
                vnext[0] += 1
            for job in out_queue:
                emit_outproj(job)
            for job in (12, 13, 14, 15):
                emit_outproj(job, tail=True)
            if debug_taps:
                nc.sync.dma_start(out=tap_qT[:], in_=qT_sb[:])
                nc.sync.dma_start(out=tap_kT[:], in_=kT_sb[:])
                nc.sync.dma_start(out=tap_v[:], in_=v_sb[:])
                for q in (0, 1):
                    nc.sync.dma_start(out=tap_xh[q][:], in_=xh_sb[q][:])

    nc.compile()
    return nc


_CACHE = {}


def _get_program(with_bv: bool):
    if with_bv not in _CACHE:
        _CACHE[with_bv] = build_program(with_bv)
    return _CACHE[with_bv]


def make_in_maps(query, key, value, mask, Wq, bq, Wk, bk, Wv, bv, Wo, bo):
    bf = ml_dtypes.bfloat16
    # transposed bf16 activations are shared by the 4 cores of each batch
    xt = {}
    for nm, x in (("xq", query), ("xk", key), ("xv", value)):
        for b in range(B):
            a = np.ascontiguousarray(x[b].T.reshape(DCH, 128, T)).astype(bf)
            if nm in ("xq", "xv"):
                # [DCH, 128, T] -> [NQB, DCH, 128, 512] (query-block major)
                a = np.ascontiguousarray(
                    a.reshape(DCH, 128, NQB, 512).transpose(2, 0, 1, 3))
            xt[nm, b] = a
    in_maps = []
    for c in range(NCORES):
        b, g = c // 4, c % 4
        cols = slice(GC * g, GC * (g + 1))
        m = {}
        for nm in ("xq", "xk", "xv"):
            m[nm] = xt[nm, b]
        for nm, W in (("wq", Wq), ("wk", Wk)):
            a = np.ascontiguousarray(
                W[cols, :].T.reshape(DCH, 128, GC).transpose(1, 0, 2)
            ).astype(bf)  # [128, DCH, GC]
            # -> [128, 2(m), DCH, 128]
            m[nm] = np.ascontiguousarray(
                a.reshape(128, DCH, 2, 128).transpose(0, 2, 1, 3))
        m["wv"] = np.ascontiguousarray(
            Wv[cols, :].T.reshape(DCH, 128, GC).transpose(1, 0, 2)).astype(bf)
        # [64(dk), 2(hh), 2(m), D]: wo2[p, hh, m, d] = Wo[d, cols[(2m+hh)*64+p]]
        m["wo"] = np.ascontiguousarray(
            Wo[:, cols].T.reshape(2, 2, 64, D).transpose(2, 1, 0, 3)
        ).astype(bf)
        mb = np.where(mask[b, 0] != 0, 0.0, MASK_NEG).astype(np.float32)
        m["maskb"] = np.ascontiguousarray(mb.reshape(KC, 128).T)
        m["bq"] = np.ascontiguousarray(
            bq[cols].reshape(2, 128).T.astype(np.float32))
        m["bk"] = np.ascontiguousarray(
            bk[cols].reshape(2, 128).T.astype(np.float32))
        m["bv"] = np.ascontiguousarray(
            bv[cols].reshape(GH, 64).T.astype(np.float32))
        in_maps.append(m)
    return in_maps


def kernel(query, key, value, mask, Wq, bq, Wk, bk, Wv, bv, Wo, bo,
           _trace=False):
    query, key, value = (np.asarray(a, np.float32) for a in (query, key, value))
    mask = np.asarray(mask)
    with_bv = bool(np.any(np.asarray(bv)))
    nc = _get_program(with_bv)
    in_maps = make_in_maps(query, key, value, mask, Wq, bq, Wk, bk, Wv, bv,
                           Wo, bo)
    res = run_bass_kernel_spmd(nc, in_maps, list(range(NCORES)), trace=_trace)
    out = np.zeros((B, T, D), np.float32)
    for c in range(NCORES):
        out[c // 4] += np.asarray(res.results[c]["out"], np.float32).T
    out += np.asarray(bo, np.float32)[None, None, :]
    if _trace:
        kernel.last_exec_time_ns = res.exec_time_ns
        kernel.last_results = res
    return out


# revision 34
# speedup vs baseline: 1.0595x; 1.0595x over previous
"""MultiHeadedAttention Trainium2 kernel (v2: slot-pipelined).

Problem: B=2, T=2048, D=1024, H=16 heads (DK=64), fp32 in/out, padding mask
on keys. out = softmax(mask(QWq (KWk)^T / 8)) @ (VWv) @ Wo^T + biases.

Sharding (8 cores): core c -> batch b = c//4, head group g = c%4 (4 heads,
256 projection columns). Each core computes its heads' attention and a
partial output projection; host sums the 4 partials per batch (+ bo).

v2 design (vs baseline):
  - Flat 128-slot pipeline: 8 units (4 query-blocks of 512 x 2 head-pairs)
    x 16 key-chunks.  Per slot: 2 concurrent score matmuls (row-tiled at
    partitions 0/64, K=64 each), ONE exp activation N=1024 covering both
    heads, and ~2 lagged attn@V matmul pairs.  ScalarE (exp) is the pacer
    at ~1.15us/slot; everything else hides underneath.
  - PSUM budget = exactly 8 banks: score ping-pong 2x[128,2,512]f32 (4),
    o2 accumulator [65,2,512]f32 (2), borrow slot [128,1024]f32 (2).
  - Projections (beyond a minimal startup prefix), v-projection and the
    output projection are emitted as "borrows" of the borrow slot, woven
    between slots at statically scheduled points.
  - Startup prefix: kT(m0) + qT(m0, qblock0) only -> first exp at ~17us
    (baseline: 36us).
  - attn@V lags exp by ~3 slots in steady state (elastic early on while
    xv/wv DMAs land), so the tail after the last exp is tiny.
  - Output DMA'd as bf16 (host sums partials in fp32): halves out traffic.
"""

import numpy as np
import ml_dtypes

import concourse.bass as bass
import concourse.bacc as bacc
import concourse.tile as tile
from concourse import mybir
from concourse.bass_utils import run_bass_kernel_spmd

B, T, D, H = 2, 2048, 1024, 16
DK = D // H  # 64
GH = 4       # heads per core
GC = GH * DK  # 256 proj columns per core
NCORES = 8
KC = T // 128   # 16 key chunks
DCH = D // 128  # 8 contraction chunks
NQB = 4         # query blocks of 512
NU = 2 * NQB    # units: u = 2*qb + pr
NSLOT = NU * KC
F32 = mybir.dt.float32
BF16 = mybir.dt.bfloat16

MASK_NEG = -30000.0


def build_program(with_bv: bool, debug_taps: bool = False):
    nc = bacc.Bacc("TRN2")

    # ---- DRAM parameters (per-core shapes) ----
    xq_d = nc.declare_dram_parameter("xq", [NQB, DCH, 128, 512], BF16,
                                     isOutput=False)
    xk_d = nc.declare_dram_parameter("xk", [DCH, 128, T], BF16, isOutput=False)
    xv_d = nc.declare_dram_parameter("xv", [NQB, DCH, 128, 512], BF16,
                                     isOutput=False)
    wq_d = nc.declare_dram_parameter("wq", [128, 2, DCH, 128], BF16,
                                     isOutput=False)
    wk_d = nc.declare_dram_parameter("wk", [128, 2, DCH, 128], BF16,
                                     isOutput=False)
    wv_d = nc.declare_dram_parameter("wv", [128, DCH, GC], BF16, isOutput=False)
    wo_d = nc.declare_dram_parameter("wo", [64, 2, 2, D], BF16,
                                     isOutput=False)
    mask_d = nc.declare_dram_parameter("maskb", [128, KC], F32, isOutput=False)
    bq_d = nc.declare_dram_parameter("bq", [128, 2], F32, isOutput=False)
    bk_d = nc.declare_dram_parameter("bk", [128, 2], F32, isOutput=False)
    bv_d = nc.declare_dram_parameter("bv", [64, GH], F32, isOutput=False)
    # transposed [D, T]: the host transposes back.  This keeps xh as the
    # MOVING operand of the output projection (wo is stationary), which
    # keeps the xh consumer robustly ordered after the norm writes.
    out_d = nc.declare_dram_parameter("out", [D, T], BF16, isOutput=True)
    if debug_taps:
        tap_qT = nc.declare_dram_parameter("tap_qT", [128, 2, T], BF16,
                                           isOutput=True)
        tap_kT = nc.declare_dram_parameter("tap_kT", [128, 2, T], BF16,
                                           isOutput=True)
        tap_v = nc.declare_dram_parameter("tap_v", [128, KC, GH, 66], BF16,
                                          isOutput=True)
        tap_xh = [nc.declare_dram_parameter(f"tap_xh{q}", [64, 2, 2, 1024],
                                            BF16, isOutput=True)
                  for q in (0, 1)]

    EXPF = mybir.ActivationFunctionType.Exp

    with tile.TileContext(nc) as tc:
        with (
            tc.tile_pool(name="persist", bufs=1) as pp,
            tc.tile_pool(name="psum", bufs=1, space="PSUM") as psp,
            tc.tile_pool(name="expool", bufs=17) as exp_pool,
            tc.tile_pool(name="normp", bufs=1) as norm_pool,
            tc.tile_pool(name="outp", bufs=2) as out_pool,
        ):
            # persistent sbuf tensors
            wq_sb = pp.tile([128, 2, DCH, 128], BF16, tag="wq")
            wk_sb = pp.tile([128, 2, DCH, 128], BF16, tag="wk")
            wv_sb = pp.tile([128, DCH, GC], BF16, tag="wv")
            wo_sb = pp.tile([64, 2, 2, D], BF16, tag="wo")
            mask_sb = pp.tile([128, KC], F32, tag="mask")
            bq_sb = pp.tile([128, 2], F32, tag="bq")
            bk_sb = pp.tile([128, 2], F32, tag="bk")
            bv_sb = pp.tile([64, GH], F32, tag="bv")
            qT_sb = pp.tile([128, 2, T], BF16, tag="qT")
            kT_sb = pp.tile([128, 2, T], BF16, tag="kT")
            v_sb = pp.tile([128, KC, GH, 66], BF16, tag="v")
            # all heads' normalized outputs live at partitions 0:64
            # ([hh, pr] in the free dims) so every write is a plain DVE op
            # and the out-projection contracts K=64 per (hh, m) pair.
            xh_sb = [pp.tile([64, 2, 2, 1024], BF16, tag=f"xh{q}",
                             name=f"xh{q}") for q in (0, 1)]
            xk_sb = [pp.tile([128, T], BF16, tag=f"xk{k}", name=f"xk{k}")
                     for k in range(DCH)]
            xq_sb = [pp.tile([128, T], BF16, tag=f"xq{k}", name=f"xq{k}")
                     for k in range(DCH)]
            xv_sb = [pp.tile([128, T], BF16, tag=f"xv{k}", name=f"xv{k}")
                     for k in range(DCH)]
            nc.vector.memset(v_sb[:, :, :, 64:65], 1.0)

            # dummy exp to pull the ACT table load into the DMA-wait window
            dmy = pp.tile([128, 16], F32, tag="dmy")
            dmy2 = pp.tile([128, 16], BF16, tag="dmy2")
            nc.vector.memset(dmy[:], 0.0)
            nc.scalar.activation(dmy2[:], dmy[:], EXPF)

            # ---- DMA schedule (issue order matters: queue is FIFO) ----
            nc.sync.dma_start(out=wk_sb[:, 0], in_=wk_d[:, 0])
            nc.sync.dma_start(out=wq_sb[:, 0], in_=wq_d[:, 0])
            nc.sync.dma_start(out=mask_sb[:], in_=mask_d[:])
            nc.sync.dma_start(out=bk_sb[:], in_=bk_d[:])
            nc.sync.dma_start(out=bq_sb[:], in_=bq_d[:])
            for k in range(DCH):
                nc.sync.dma_start(out=xk_sb[k][:], in_=xk_d[k])
            for k in range(DCH):  # q block 0
                nc.sync.dma_start(out=xq_sb[k][:, 0:512], in_=xq_d[0, k])
            nc.sync.dma_start(out=wk_sb[:, 1], in_=wk_d[:, 1])
            nc.sync.dma_start(out=wq_sb[:, 1], in_=wq_d[:, 1])
            nc.sync.dma_start(out=wv_sb[:], in_=wv_d[:])
            for k in range(DCH):  # v block 0
                nc.sync.dma_start(out=xv_sb[k][:, 0:512], in_=xv_d[0, k])
            for qb in (1, 2):
                for k in range(DCH):
                    nc.sync.dma_start(out=xq_sb[k][:, qb * 512:(qb + 1) * 512],
                                      in_=xq_d[qb, k])
                for k in range(DCH):
                    nc.sync.dma_start(out=xv_sb[k][:, qb * 512:(qb + 1) * 512],
                                      in_=xv_d[qb, k])
            for k in range(DCH):
                nc.sync.dma_start(out=xq_sb[k][:, 3 * 512:4 * 512],
                                  in_=xq_d[3, k])
            nc.sync.dma_start(out=wo_sb[:], in_=wo_d[:])
            for k in range(DCH):
                nc.sync.dma_start(out=xv_sb[k][:, 3 * 512:4 * 512],
                                  in_=xv_d[3, k])
            nc.sync.dma_start(out=bv_sb[:], in_=bv_d[:])

            # ---- helpers ----
            def emit_proj(dst, m, w_sb, x_sb, c0, width, b_sb, tag):
                """dst = (W_m^T x)[:, c0:c0+width] + bias_m."""
                pst = psp.tile([128, 1024], F32, tag=tag,
                               bufs=1 if tag in ("br", "o2") else 2,
                               name="pst")
                nhalf = width // 512
                for k in range(DCH):
                    for n in range(nhalf):
                        nc.tensor.matmul(
                            pst[:, n * 512:(n + 1) * 512],
                            w_sb[:, m, k, :],
                            x_sb[k][:, c0 + n * 512:c0 + (n + 1) * 512],
                            start=(k == 0), stop=(k == DCH - 1),
                            skip_group_check=True,
                        )
                nc.vector.tensor_scalar_add(
                    dst[:], pst[:, 0:width], b_sb[:, m:m + 1])

            def emit_vproj(g):
                """v projection for token chunks 4g..4g+3 (= key chunks)."""
                vps = psp.tile([128, 4, GH, 64], F32, tag="br", bufs=1,
                               name="vps")
                for t in range(4):
                    tcn = 4 * g + t
                    for k in range(DCH):
                        nc.tensor.matmul(
                            vps[:, t, :, :],
                            xv_sb[k][:, tcn * 128:(tcn + 1) * 128],
                            wv_sb[:, k, :],
                            start=(k == 0), stop=(k == DCH - 1),
                            skip_group_check=True,
                        )
                nc.vector.tensor_copy(v_sb[:, 4 * g:4 * g + 4, :, 0:64],
                                      vps[:])

            def emit_outproj(job, tail=False, tag="br"):
                """partial out^T for token block qb, d-chunks 2*dcg..2*dcg+1.

                po[dout, t] = sum_m sum_p wo[p, m, dout] * xh[p, m, t]:
                wo is the stationary operand, xh the moving one."""
                qb, dcg = job // 4, job % 4
                qh, off = qb // 2, (qb % 2) * 512
                po = psp.tile([128, 2, 512], F32, tag=tag,
                              bufs=1 if tag in ("br", "o2") else 2, name="po")
                for d2 in range(2):
                    dc = 2 * dcg + d2
                    for i, (m, hh) in enumerate(
                            ((0, 0), (0, 1), (1, 0), (1, 1))):
                        nc.tensor.matmul(
                            po[:, d2, :],
                            wo_sb[0:64, hh, m, dc * 128:(dc + 1) * 128],
                            xh_sb[qh][0:64, hh, m, off:off + 512],
                            start=(i == 0), stop=(i == 3),
                            skip_group_check=True,
                        )
                ot = out_pool.tile([128, 2, 512], BF16, tag="ot")
                if tail and job % 2 == 0:
                    nc.scalar.copy(ot[:], po[:])
                else:
                    nc.vector.tensor_copy(ot[:], po[:])
                for d2 in range(2):
                    dc = 2 * dcg + d2
                    nc.sync.dma_start(
                        out=out_d[dc * 128:(dc + 1) * 128,
                                  qb * 512:(qb + 1) * 512],
                        in_=ot[:, d2, :])

            def emit_norm(u, o2):
                """normalize o2 -> xh for unit u=(qb,pr)."""
                qb, pr = u // 2, u % 2
                qh, off = qb // 2, (qb % 2) * 512
                rd = norm_pool.tile([1, 2, 512], F32, tag="rd", name="rd")
                rc = norm_pool.tile([1, 2, 512], F32, tag="rc", name="rc")
                nc.vector.tensor_copy(rd[:], o2[64:65, :, :])
                nc.vector.reciprocal_approx_fast(rc[:], rd[:])
                rb = norm_pool.tile([64, 2, 512], F32, tag="rb", name="rb")
                nc.gpsimd.partition_broadcast(rb[:], rc[:])
                for hh in range(2):
                    dst = xh_sb[qh][0:64, hh, pr, off:off + 512]
                    nc.vector.tensor_mul(dst, o2[0:64, hh, :], rb[:, hh, :])
                    if with_bv:
                        nc.vector.tensor_scalar_add(
                            dst, dst, bv_sb[:, 2 * pr + hh:2 * pr + hh + 1])

            # ---- startup prefix: kT(m0) full + qT(m0, qb0) ----
            # the two 1024-key halves are interleaved chunk-by-chunk so the
            # matmuls trail the xk DMA stream with no serialization.
            pst0 = psp.tile([128, 1024], F32, tag="br", bufs=1, name="pst0")
            pst1 = psp.tile([128, 1024], F32, tag="sc", bufs=2, name="pst1")
            for k in range(DCH):
                for half, pst in ((0, pst0), (1, pst1)):
                    for n in range(2):
                        c = half * 1024 + n * 512
                        nc.tensor.matmul(
                            pst[:, n * 512:(n + 1) * 512],
                            wk_sb[:, 0, k, :], xk_sb[k][:, c:c + 512],
                            start=(k == 0), stop=(k == DCH - 1),
                            skip_group_check=True,
                        )
            nc.vector.tensor_scalar_add(kT_sb[:, 0, 0:1024], pst0[:],
                                        bk_sb[:, 0:1])
            nc.vector.tensor_scalar_add(kT_sb[:, 0, 1024:2048], pst1[:],
                                        bk_sb[:, 0:1])
            emit_proj(qT_sb[:, 0, 0:512], 0, wq_sb, xq_sb, 0, 512, bq_sb,
                      "sc")

            # ---- borrow plan: slot index -> list of thunks ----
            plan = {}

            def at(s, fn, *a, **kw):
                plan.setdefault(s, []).append(lambda: fn(*a, **kw))

            at(3, emit_proj, qT_sb[:, 1, 0:512], 1, wq_sb, xq_sb, 0, 512,
               bq_sb, "br")
            at(7, emit_proj, kT_sb[:, 1, 0:1024], 1, wk_sb, xk_sb, 0, 1024,
               bk_sb, "br")
            at(11, emit_proj, kT_sb[:, 1, 1024:2048], 1, wk_sb, xk_sb, 1024,
               1024, bk_sb, "br")
            at(15, emit_vproj, 0)
            at(19, emit_vproj, 1)
            at(23, emit_proj, qT_sb[:, 0, 512:1024], 0, wq_sb, xq_sb, 512,
               512, bq_sb, "br")
            at(27, emit_vproj, 2)
            at(31, emit_vproj, 3)
            at(35, emit_proj, qT_sb[:, 1, 512:1024], 1, wq_sb, xq_sb, 512,
               512, bq_sb, "br")
            # late projections go late: they both fill the underutilized
            # back half (keeps HAM from throttling) and meet their deadlines
            # (qb2 scores need m0 by s64/m1 by s80; qb3 by s96/s112).
            at(43, emit_proj, qT_sb[:, 0, 1024:1536], 0, wq_sb, xq_sb, 1024,
               512, bq_sb, "br")
            at(58, emit_proj, qT_sb[:, 1, 1024:1536], 1, wq_sb, xq_sb, 1024,
               512, bq_sb, "br")
            at(80, emit_proj, qT_sb[:, 0, 1536:2048], 0, wq_sb, xq_sb, 1536,
               512, bq_sb, "br")
            at(95, emit_proj, qT_sb[:, 1, 1536:2048], 1, wq_sb, xq_sb, 1536,
               512, bq_sb, "br")
            # outproj jobs 0-11 are emitted via a readiness-gated queue (the
            # job must come AFTER both of its units' norms in program order,
            # or Tile sees a read-before-write and drops the dependency).
            # qb3 (jobs 12-15) must wait for norm(u7): tail only.
            out_min_slot = {0: 49, 1: 52, 2: 55, 3: 62,
                            4: 68, 5: 72, 6: 76, 7: 86,
                            8: 101, 9: 106, 10: 111, 11: 116}

            # vproj group g emitted at slot:
            vproj_slot = {0: 15, 1: 19, 2: 27, 3: 31}

            # ---- the slot loop ----
            ex_tiles = {}
            o2_cur = [None]
            vnext = [0]
            VLAG = 3
            norm_slot = {}      # unit -> slot its norm was emitted at
            out_queue = list(range(12))

            def emit_V(t, s):
                u, kc = t // KC, t % KC
                pr = u % 2
                if kc == 0:
                    o2_cur[0] = psp.tile([65, 2, 512], F32, tag="o2", bufs=1,
                                         name="o2")
                o2 = o2_cur[0]
                for hh in range(2):
                    nc.tensor.matmul(
                        o2[:, hh, :],
                        v_sb[:, kc, 2 * pr + hh, 0:65],
                        ex_tiles[t][:, hh, :],
                        start=(kc == 0), stop=(kc == KC - 1),
                        skip_group_check=True,
                    )
                if kc == KC - 1:
                    emit_norm(u, o2)
                    norm_slot[u] = s
                del ex_tiles[t]

            def v_ready(t, s):
                if t > s - VLAG:
                    return False
                if vproj_slot[(t % KC) // 4] + 5 > s:
                    return False
                return True

            for s in range(NSLOT):
                u, kc = s // KC, s % KC
                qb, pr = u // 2, u % 2
                sc = psp.tile([128, 2, 512], F32, tag="sc", bufs=2, name="sc")
                for hh in range(2):
                    nc.tensor.matmul(
                        sc[:, hh, :],
                        kT_sb[64 * hh:64 * hh + 64, pr,
                              kc * 128:(kc + 1) * 128],
                        qT_sb[64 * hh:64 * hh + 64, pr,
                              qb * 512:(qb + 1) * 512],
                        start=True, stop=True,
                    )
                ex = exp_pool.tile([128, 2, 512], BF16, tag="ex", name="ex")
                nc.scalar.activation(ex[:], sc[:], EXPF,
                                     bias=mask_sb[:, kc:kc + 1],
                                     scale=float(DK) ** -0.5)
                ex_tiles[s] = ex
                # lagged attn@V (up to 3 slot-pairs per slot)
                nv = 0
                while vnext[0] < NSLOT and nv < 3 and v_ready(vnext[0], s):
                    emit_V(vnext[0], s)
                    vnext[0] += 1
                    nv += 1
                for fn in plan.get(s, []):
                    fn()
                while out_queue:
                    job = out_queue[0]
                    qb = job // 4
                    us = (2 * qb, 2 * qb + 1)
                    if (s >= out_min_slot[job]
                            and all(norm_slot.get(u, 999) <= s - 2
                                    for u in us)):
                        emit_outproj(out_queue.pop(0))
                    else:
                        break

            # ---- tail: drain V, final norm, qb3 outproj ----
            # alternate between the br and sc PSUM tags so two output jobs
            # are in flight (the score ping-pong buffers are free by now)
            while vnext[0] < NSLOT:
                emit_V(vnext[0], NSLOT + 8)
                vnext[0] += 1
            tail_jobs = list(out_queue) + [12, 13, 14, 15]
            for i, job in enumerate(tail_jobs):
                emit_outproj(job, tail=True, tag=("br", "sc")[i % 2])
            if debug_taps:
                nc.sync.dma_start(out=tap_qT[:], in_=qT_sb[:])
                nc.sync.dma_start(out=tap_kT[:], in_=kT_sb[:])
                nc.sync.dma_start(out=tap_v[:], in_=v_sb[:])
                for q in (0, 1):
                    nc.sync.dma_start(out=tap_xh[q][:], in_=xh_sb[q][:])

    nc.compile()
    return nc


_CACHE = {}


def _get_program(with_bv: bool):
    if with_bv not in _CACHE:
        _CACHE[with_bv] = build_program(with_bv)
    return _CACHE[with_bv]


def make_in_maps(query, key, value, mask, Wq, bq, Wk, bk, Wv, bv, Wo, bo):
    bf = ml_dtypes.bfloat16
    # transposed bf16 activations are shared by the 4 cores of each batch
    xt = {}
    for nm, x in (("xq", query), ("xk", key), ("xv", value)):
        for b in range(B):
            a = np.ascontiguousarray(x[b].T.reshape(DCH, 128, T)).astype(bf)
            if nm in ("xq", "xv"):
                # [DCH, 128, T] -> [NQB, DCH, 128, 512] (query-block major)
                a = np.ascontiguousarray(
                    a.reshape(DCH, 128, NQB, 512).transpose(2, 0, 1, 3))
            xt[nm, b] = a
    in_maps = []
    for c in range(NCORES):
        b, g = c // 4, c % 4
        cols = slice(GC * g, GC * (g + 1))
        m = {}
        for nm in ("xq", "xk", "xv"):
            m[nm] = xt[nm, b]
        for nm, W in (("wq", Wq), ("wk", Wk)):
            a = np.ascontiguousarray(
                W[cols, :].T.reshape(DCH, 128, GC).transpose(1, 0, 2)
            ).astype(bf)  # [128, DCH, GC]
            # -> [128, 2(m), DCH, 128]
            m[nm] = np.ascontiguousarray(
                a.reshape(128, DCH, 2, 128).transpose(0, 2, 1, 3))
        m["wv"] = np.ascontiguousarray(
            Wv[cols, :].T.reshape(DCH, 128, GC).transpose(1, 0, 2)).astype(bf)
        # [64(dk), 2(hh), 2(m), D]: wo2[p, hh, m, d] = Wo[d, cols[(2m+hh)*64+p]]
        m["wo"] = np.ascontiguousarray(
            Wo[:, cols].T.reshape(2, 2, 64, D).transpose(2, 1, 0, 3)
        ).astype(bf)
        mb = np.where(mask[b, 0] != 0, 0.0, MASK_NEG).astype(np.float32)
        m["maskb"] = np.ascontiguousarray(mb.reshape(KC, 128).T)
        m["bq"] = np.ascontiguousarray(
            bq[cols].reshape(2, 128).T.astype(np.float32))
        m["bk"] = np.ascontiguousarray(
            bk[cols].reshape(2, 128).T.astype(np.float32))
        m["bv"] = np.ascontiguousarray(
            bv[cols].reshape(GH, 64).T.astype(np.float32))
        in_maps.append(m)
    return in_maps


def kernel(query, key, value, mask, Wq, bq, Wk, bk, Wv, bv, Wo, bo,
           _trace=False):
    query, key, value = (np.asarray(a, np.float32) for a in (query, key, value))
    mask = np.asarray(mask)
    with_bv = bool(np.any(np.asarray(bv)))
    nc = _get_program(with_bv)
    in_maps = make_in_maps(query, key, value, mask, Wq, bq, Wk, bk, Wv, bv,
                           Wo, bo)
    res = run_bass_kernel_spmd(nc, in_maps, list(range(NCORES)), trace=_trace)
    out = np.zeros((B, T, D), np.float32)
    for c in range(NCORES):
        out[c // 4] += np.asarray(res.results[c]["out"], np.float32).T
    out += np.asarray(bo, np.float32)[None, None, :]
    if _trace:
        kernel.last_exec_time_ns = res.exec_time_ns
        kernel.last_results = res
    return out


# revision 43
# speedup vs baseline: 1.1031x; 1.0411x over previous
"""MultiHeadedAttention Trainium2 kernel (v2: slot-pipelined).

Problem: B=2, T=2048, D=1024, H=16 heads (DK=64), fp32 in/out, padding mask
on keys. out = softmax(mask(QWq (KWk)^T / 8)) @ (VWv) @ Wo^T + biases.

Sharding (8 cores): core c -> batch b = c//4, head group g = c%4 (4 heads,
256 projection columns). Each core computes its heads' attention and a
partial output projection; host sums the 4 partials per batch (+ bo).

v2 design (vs baseline):
  - Flat 128-slot pipeline: 8 units (4 query-blocks of 512 x 2 head-pairs)
    x 16 key-chunks.  Per slot: 2 concurrent score matmuls (row-tiled at
    partitions 0/64, K=64 each), ONE exp activation N=1024 covering both
    heads, and ~2 lagged attn@V matmul pairs.  ScalarE (exp) is the pacer
    at ~1.15us/slot; everything else hides underneath.
  - PSUM budget = exactly 8 banks: score ping-pong 2x[128,2,512]f32 (4),
    o2 accumulator [65,2,512]f32 (2), borrow slot [128,1024]f32 (2).
  - Projections (beyond a minimal startup prefix), v-projection and the
    output projection are emitted as "borrows" of the borrow slot, woven
    between slots at statically scheduled points.
  - Startup prefix: kT(m0) + qT(m0, qblock0) only -> first exp at ~17us
    (baseline: 36us).
  - attn@V lags exp by ~3 slots in steady state (elastic early on while
    xv/wv DMAs land), so the tail after the last exp is tiny.
  - Output DMA'd as bf16 (host sums partials in fp32): halves out traffic.
"""

import numpy as np
import ml_dtypes

import concourse.bass as bass
import concourse.bacc as bacc
import concourse.tile as tile
from concourse import mybir
from concourse.bass_utils import run_bass_kernel_spmd

B, T, D, H = 2, 2048, 1024, 16
DK = D // H  # 64
GH = 4       # heads per core
GC = GH * DK  # 256 proj columns per core
NCORES = 8
KC = T // 128   # 16 key chunks
DCH = D // 128  # 8 contraction chunks
NQB = 4         # query blocks of 512
NU = 2 * NQB    # units: u = 2*qb + pr
NSLOT = NU * KC
F32 = mybir.dt.float32
BF16 = mybir.dt.bfloat16

MASK_NEG = -30000.0


def build_program(with_bv: bool, debug_taps: bool = False):
    nc = bacc.Bacc("TRN2")

    # ---- DRAM parameters (per-core shapes) ----
    xq_d = nc.declare_dram_parameter("xq", [NQB, DCH, 128, 512], BF16,
                                     isOutput=False)
    xk_d = nc.declare_dram_parameter("xk", [DCH, 128, T], BF16, isOutput=False)
    xv_d = nc.declare_dram_parameter("xv", [NQB, DCH, 128, 512], BF16,
                                     isOutput=False)
    wq_d = nc.declare_dram_parameter("wq", [128, 2, DCH, 128], BF16,
                                     isOutput=False)
    wk_d = nc.declare_dram_parameter("wk", [128, 2, DCH, 128], BF16,
                                     isOutput=False)
    wv_d = nc.declare_dram_parameter("wv", [128, DCH, GC], BF16, isOutput=False)
    wo_d = nc.declare_dram_parameter("wo", [128, 2, D], BF16, isOutput=False)
    mask_d = nc.declare_dram_parameter("maskb", [128, KC], F32, isOutput=False)
    bq_d = nc.declare_dram_parameter("bq", [128, 2], F32, isOutput=False)
    bk_d = nc.declare_dram_parameter("bk", [128, 2], F32, isOutput=False)
    bv_d = nc.declare_dram_parameter("bv", [64, GH], F32, isOutput=False)
    # transposed [D, T]: the host transposes back.  This keeps xh as the
    # MOVING operand of the output projection (wo is stationary), which
    # keeps the xh consumer robustly ordered after the norm writes.
    out_d = nc.declare_dram_parameter("out", [D, T], BF16, isOutput=True)
    if debug_taps:
        tap_qT = nc.declare_dram_parameter("tap_qT", [128, 2, T], BF16,
                                           isOutput=True)
        tap_kT = nc.declare_dram_parameter("tap_kT", [128, 2, T], BF16,
                                           isOutput=True)
        tap_v = nc.declare_dram_parameter("tap_v", [128, KC, GH, 66], BF16,
                                          isOutput=True)
        tap_xh = [nc.declare_dram_parameter(f"tap_xh{q}", [128, 2, 1024],
                                            BF16, isOutput=True)
                  for q in (0, 1)]

    EXPF = mybir.ActivationFunctionType.Exp

    with tile.TileContext(nc) as tc:
        with (
            tc.tile_pool(name="persist", bufs=1) as pp,
            tc.tile_pool(name="psum", bufs=1, space="PSUM") as psp,
            tc.tile_pool(name="expool", bufs=20) as exp_pool,
            tc.tile_pool(name="normp", bufs=1) as norm_pool,
            tc.tile_pool(name="outp", bufs=2) as out_pool,
        ):
            # persistent sbuf tensors
            wq_sb = pp.tile([128, 2, DCH, 128], BF16, tag="wq")
            wk_sb = pp.tile([128, 2, DCH, 128], BF16, tag="wk")
            wv_sb = pp.tile([128, DCH, GC], BF16, tag="wv")
            wo_sb = pp.tile([128, 2, D], BF16, tag="wo")
            mask_sb = pp.tile([128, KC], F32, tag="mask")
            bq_sb = pp.tile([128, 2], F32, tag="bq")
            bk_sb = pp.tile([128, 2], F32, tag="bk")
            bv_sb = pp.tile([64, GH], F32, tag="bv")
            qT_sb = pp.tile([128, 2, T], BF16, tag="qT")
            kT_sb = pp.tile([128, 2, T], BF16, tag="kT")
            v_sb = pp.tile([128, KC, GH, 66], BF16, tag="v")
            xh_sb = [pp.tile([128, 2, 1024], BF16, tag=f"xh{q}",
                             name=f"xh{q}") for q in (0, 1)]
            xk_sb = [pp.tile([128, T], BF16, tag=f"xk{k}", name=f"xk{k}")
                     for k in range(DCH)]
            xq_sb = [pp.tile([128, T], BF16, tag=f"xq{k}", name=f"xq{k}")
                     for k in range(DCH)]
            xv_sb = [pp.tile([128, T], BF16, tag=f"xv{k}", name=f"xv{k}")
                     for k in range(DCH)]
            nc.vector.memset(v_sb[:, :, :, 64:65], 1.0)

            # dummy exp to pull the ACT table load into the DMA-wait window
            dmy = pp.tile([128, 16], F32, tag="dmy")
            dmy2 = pp.tile([128, 16], BF16, tag="dmy2")
            nc.vector.memset(dmy[:], 0.0)
            nc.scalar.activation(dmy2[:], dmy[:], EXPF)

            # ---- DMA schedule (issue order matters: queue is FIFO) ----
            nc.sync.dma_start(out=wk_sb[:, 0], in_=wk_d[:, 0])
            nc.sync.dma_start(out=wq_sb[:, 0], in_=wq_d[:, 0])
            nc.sync.dma_start(out=mask_sb[:], in_=mask_d[:])
            nc.sync.dma_start(out=bk_sb[:], in_=bk_d[:])
            nc.sync.dma_start(out=bq_sb[:], in_=bq_d[:])
            for k in range(DCH):
                nc.sync.dma_start(out=xk_sb[k][:], in_=xk_d[k])
            for k in range(DCH):  # q block 0
                nc.sync.dma_start(out=xq_sb[k][:, 0:512], in_=xq_d[0, k])
            nc.sync.dma_start(out=wk_sb[:, 1], in_=wk_d[:, 1])
            nc.sync.dma_start(out=wq_sb[:, 1], in_=wq_d[:, 1])
            nc.sync.dma_start(out=wv_sb[:], in_=wv_d[:])
            for k in range(DCH):  # v block 0
                nc.sync.dma_start(out=xv_sb[k][:, 0:512], in_=xv_d[0, k])
            for qb in (1, 2):
                for k in range(DCH):
                    nc.sync.dma_start(out=xq_sb[k][:, qb * 512:(qb + 1) * 512],
                                      in_=xq_d[qb, k])
                for k in range(DCH):
                    nc.sync.dma_start(out=xv_sb[k][:, qb * 512:(qb + 1) * 512],
                                      in_=xv_d[qb, k])
            for k in range(DCH):
                nc.sync.dma_start(out=xq_sb[k][:, 3 * 512:4 * 512],
                                  in_=xq_d[3, k])
            nc.sync.dma_start(out=wo_sb[:], in_=wo_d[:])
            for k in range(DCH):
                nc.sync.dma_start(out=xv_sb[k][:, 3 * 512:4 * 512],
                                  in_=xv_d[3, k])
            nc.sync.dma_start(out=bv_sb[:], in_=bv_d[:])

            # ---- helpers ----
            def emit_proj(dst, m, w_sb, x_sb, c0, width, b_sb, tag):
                """dst = (W_m^T x)[:, c0:c0+width] + bias_m."""
                pst = psp.tile([128, 1024], F32, tag=tag,
                               bufs=1 if tag in ("br", "o2") else 2,
                               name="pst")
                nhalf = width // 512
                for k in range(DCH):
                    for n in range(nhalf):
                        nc.tensor.matmul(
                            pst[:, n * 512:(n + 1) * 512],
                            w_sb[:, m, k, :],
                            x_sb[k][:, c0 + n * 512:c0 + (n + 1) * 512],
                            start=(k == 0), stop=(k == DCH - 1),
                            skip_group_check=True,
                        )
                nc.vector.tensor_scalar_add(
                    dst[:], pst[:, 0:width], b_sb[:, m:m + 1])

            def emit_vproj(g):
                """v projection for token chunks 4g..4g+3 (= key chunks)."""
                vps = psp.tile([128, 4, GH, 64], F32, tag="br", bufs=1,
                               name="vps")
                for t in range(4):
                    tcn = 4 * g + t
                    for k in range(DCH):
                        nc.tensor.matmul(
                            vps[:, t, :, :],
                            xv_sb[k][:, tcn * 128:(tcn + 1) * 128],
                            wv_sb[:, k, :],
                            start=(k == 0), stop=(k == DCH - 1),
                            skip_group_check=True,
                        )
                nc.vector.tensor_copy(v_sb[:, 4 * g:4 * g + 4, :, 0:64],
                                      vps[:])

            def emit_outproj(job, tail=False, tag="br"):
                """partial out^T for token block qb, d-chunks 2*dcg..2*dcg+1.

                po[dout, t] = sum_m sum_p wo[p, m, dout] * xh[p, m, t]:
                wo is the stationary operand, xh the moving one."""
                qb, dcg = job // 4, job % 4
                qh, off = qb // 2, (qb % 2) * 512
                po = psp.tile([128, 2, 512], F32, tag=tag,
                              bufs=1 if tag in ("br", "o2") else 2, name="po")
                for d2 in range(2):
                    dc = 2 * dcg + d2
                    for m in range(2):
                        nc.tensor.matmul(
                            po[:, d2, :],
                            wo_sb[:, m, dc * 128:(dc + 1) * 128],
                            xh_sb[qh][:, m, off:off + 512],
                            start=(m == 0), stop=(m == 1),
                            skip_group_check=True,
                        )
                ot = out_pool.tile([128, 2, 512], BF16, tag="ot")
                if tail and job % 2 == 0:
                    nc.scalar.copy(ot[:], po[:])
                else:
                    nc.vector.tensor_copy(ot[:], po[:])
                for d2 in range(2):
                    dc = 2 * dcg + d2
                    nc.sync.dma_start(
                        out=out_d[dc * 128:(dc + 1) * 128,
                                  qb * 512:(qb + 1) * 512],
                        in_=ot[:, d2, :])

            def emit_norm(u, o2):
                """normalize o2 -> xh for unit u=(qb,pr)."""
                qb, pr = u // 2, u % 2
                qh, off = qb // 2, (qb % 2) * 512
                rd = norm_pool.tile([1, 2, 512], F32, tag="rd", name="rd")
                rc = norm_pool.tile([1, 2, 512], F32, tag="rc", name="rc")
                nc.vector.tensor_copy(rd[:], o2[64:65, :, :])
                nc.vector.reciprocal_approx_fast(rc[:], rd[:])
                rb = norm_pool.tile([64, 2, 512], F32, tag="rb", name="rb")
                nc.gpsimd.partition_broadcast(rb[:], rc[:])
                # head hh=0 -> partitions 0:64 directly
                dst = xh_sb[qh][0:64, pr, off:off + 512]
                nc.vector.tensor_mul(dst, o2[0:64, 0, :], rb[:, 0, :])
                if with_bv:
                    nc.vector.tensor_scalar_add(
                        dst, dst, bv_sb[:, 2 * pr:2 * pr + 1])
                # head hh=1 -> partitions 64:128 via tmp + SBUF->SBUF DMA
                # (safe: consumers are emission-gated after this norm)
                tmp = norm_pool.tile([64, 512], BF16, tag="tmp", name="tmp",
                                     bufs=2)
                nc.vector.tensor_mul(tmp[:], o2[0:64, 1, :], rb[:, 1, :])
                if with_bv:
                    nc.vector.tensor_scalar_add(
                        tmp[:], tmp[:], bv_sb[:, 2 * pr + 1:2 * pr + 2])
                nc.sync.dma_start(
                    out=xh_sb[qh][64:128, pr, off:off + 512], in_=tmp[:])

            # ---- startup prefix: kT(m0) full + qT(m0, qb0) ----
            # the two 1024-key halves are interleaved chunk-by-chunk so the
            # matmuls trail the xk DMA stream with no serialization.
            pst0 = psp.tile([128, 1024], F32, tag="br", bufs=1, name="pst0")
            pst1 = psp.tile([128, 1024], F32, tag="sc", bufs=2, name="pst1")
            for k in range(DCH):
                for half, pst in ((0, pst0), (1, pst1)):
                    for n in range(2):
                        c = half * 1024 + n * 512
                        nc.tensor.matmul(
                            pst[:, n * 512:(n + 1) * 512],
                            wk_sb[:, 0, k, :], xk_sb[k][:, c:c + 512],
                            start=(k == 0), stop=(k == DCH - 1),
                            skip_group_check=True,
                        )
            nc.vector.tensor_scalar_add(kT_sb[:, 0, 0:1024], pst0[:],
                                        bk_sb[:, 0:1])
            nc.vector.tensor_scalar_add(kT_sb[:, 0, 1024:2048], pst1[:],
                                        bk_sb[:, 0:1])
            emit_proj(qT_sb[:, 0, 0:512], 0, wq_sb, xq_sb, 0, 512, bq_sb,
                      "sc")

            # ---- borrow plan: slot index -> list of thunks ----
            plan = {}

            def at(s, fn, *a, **kw):
                plan.setdefault(s, []).append(lambda: fn(*a, **kw))

            at(3, emit_proj, qT_sb[:, 1, 0:512], 1, wq_sb, xq_sb, 0, 512,
               bq_sb, "br")
            at(7, emit_proj, kT_sb[:, 1, 0:1024], 1, wk_sb, xk_sb, 0, 1024,
               bk_sb, "br")
            at(11, emit_proj, kT_sb[:, 1, 1024:2048], 1, wk_sb, xk_sb, 1024,
               1024, bk_sb, "br")
            at(15, emit_vproj, 0)
            at(19, emit_vproj, 1)
            at(23, emit_proj, qT_sb[:, 0, 512:1024], 0, wq_sb, xq_sb, 512,
               512, bq_sb, "br")
            at(27, emit_vproj, 2)
            at(31, emit_vproj, 3)
            at(35, emit_proj, qT_sb[:, 1, 512:1024], 1, wq_sb, xq_sb, 512,
               512, bq_sb, "br")
            # late projections go late: they both fill the underutilized
            # back half (keeps HAM from throttling) and meet their deadlines
            # (qb2 scores need m0 by s64/m1 by s80; qb3 by s96/s112).
            at(43, emit_proj, qT_sb[:, 0, 1024:1536], 0, wq_sb, xq_sb, 1024,
               512, bq_sb, "br")
            at(58, emit_proj, qT_sb[:, 1, 1024:1536], 1, wq_sb, xq_sb, 1024,
               512, bq_sb, "br")
            at(80, emit_proj, qT_sb[:, 0, 1536:2048], 0, wq_sb, xq_sb, 1536,
               512, bq_sb, "br")
            at(95, emit_proj, qT_sb[:, 1, 1536:2048], 1, wq_sb, xq_sb, 1536,
               512, bq_sb, "br")
            # outproj jobs 0-11 are emitted via a readiness-gated queue (the
            # job must come AFTER both of its units' norms in program order,
            # or Tile sees a read-before-write and drops the dependency).
            # qb3 (jobs 12-15) must wait for norm(u7): tail only.
            out_min_slot = {0: 49, 1: 52, 2: 55, 3: 62,
                            4: 68, 5: 72, 6: 76, 7: 86,
                            8: 101, 9: 106, 10: 111, 11: 116}

            # vproj group g emitted at slot:
            vproj_slot = {0: 15, 1: 19, 2: 27, 3: 31}

            # ---- the slot loop ----
            ex_tiles = {}
            o2_cur = [None]
            vnext = [0]
            VLAG = 3
            norm_slot = {}      # unit -> slot its norm was emitted at
            out_queue = list(range(12))

            def emit_V(t, s):
                u, kc = t // KC, t % KC
                pr = u % 2
                if kc == 0:
                    o2_cur[0] = psp.tile([65, 2, 512], F32, tag="o2", bufs=1,
                                         name="o2")
                o2 = o2_cur[0]
                for hh in range(2):
                    nc.tensor.matmul(
                        o2[:, hh, :],
                        v_sb[:, kc, 2 * pr + hh, 0:65],
                        ex_tiles[t][:, hh, :],
                        start=(kc == 0), stop=(kc == KC - 1),
                        skip_group_check=True,
                    )
                if kc == KC - 1:
                    emit_norm(u, o2)
                    norm_slot[u] = s
                del ex_tiles[t]

            def v_ready(t, s):
                if t > s - VLAG:
                    return False
                if vproj_slot[(t % KC) // 4] + 5 > s:
                    return False
                return True

            for s in range(NSLOT):
                u, kc = s // KC, s % KC
                qb, pr = u // 2, u % 2
                sc = psp.tile([128, 2, 512], F32, tag="sc", bufs=2, name="sc")
                for hh in range(2):
                    nc.tensor.matmul(
                        sc[:, hh, :],
                        kT_sb[64 * hh:64 * hh + 64, pr,
                              kc * 128:(kc + 1) * 128],
                        qT_sb[64 * hh:64 * hh + 64, pr,
                              qb * 512:(qb + 1) * 512],
                        start=True, stop=True,
                    )
                ex = exp_pool.tile([128, 2, 512], BF16, tag="ex", name="ex")
                nc.scalar.activation(ex[:], sc[:], EXPF,
                                     bias=mask_sb[:, kc:kc + 1],
                                     scale=float(DK) ** -0.5)
                ex_tiles[s] = ex
                # lagged attn@V (up to 3 slot-pairs per slot)
                nv = 0
                while vnext[0] < NSLOT and nv < 3 and v_ready(vnext[0], s):
                    emit_V(vnext[0], s)
                    vnext[0] += 1
                    nv += 1
                for fn in plan.get(s, []):
                    fn()
                while out_queue:
                    job = out_queue[0]
                    qb = job // 4
                    us = (2 * qb, 2 * qb + 1)
                    if (s >= out_min_slot[job]
                            and all(norm_slot.get(u, 999) <= s - 3
                                    for u in us)):
                        emit_outproj(out_queue.pop(0))
                    else:
                        break

            # ---- tail: drain V, final norm, qb3 outproj ----
            # alternate between the br and sc PSUM tags so two output jobs
            # are in flight (the score ping-pong buffers are free by now)
            while vnext[0] < NSLOT:
                emit_V(vnext[0], NSLOT + 8)
                vnext[0] += 1
            tail_jobs = list(out_queue) + [12, 13, 14, 15]
            for i, job in enumerate(tail_jobs):
                emit_outproj(job, tail=True, tag=("br", "sc")[i % 2])
            if debug_taps:
                nc.sync.dma_start(out=tap_qT[:], in_=qT_sb[:])
                nc.sync.dma_start(out=tap_kT[:], in_=kT_sb[:])
                nc.sync.dma_start(out=tap_v[:], in_=v_sb[:])
                for q in (0, 1):
                    nc.sync.dma_start(out=tap_xh[q][:], in_=xh_sb[q][:])

    nc.compile()
    return nc


_CACHE = {}


def _get_program(with_bv: bool):
    if with_bv not in _CACHE:
        _CACHE[with_bv] = build_program(with_bv)
    return _CACHE[with_bv]


def make_in_maps(query, key, value, mask, Wq, bq, Wk, bk, Wv, bv, Wo, bo):
    bf = ml_dtypes.bfloat16
    # transposed bf16 activations are shared by the 4 cores of each batch
    xt = {}
    for nm, x in (("xq", query), ("xk", key), ("xv", value)):
        for b in range(B):
            a = np.ascontiguousarray(x[b].T.reshape(DCH, 128, T)).astype(bf)
            if nm in ("xq", "xv"):
                # [DCH, 128, T] -> [NQB, DCH, 128, 512] (query-block major)
                a = np.ascontiguousarray(
                    a.reshape(DCH, 128, NQB, 512).transpose(2, 0, 1, 3))
            xt[nm, b] = a
    in_maps = []
    for c in range(NCORES):
        b, g = c // 4, c % 4
        cols = slice(GC * g, GC * (g + 1))
        m = {}
        for nm in ("xq", "xk", "xv"):
            m[nm] = xt[nm, b]
        for nm, W in (("wq", Wq), ("wk", Wk)):
            a = np.ascontiguousarray(
                W[cols, :].T.reshape(DCH, 128, GC).transpose(1, 0, 2)
            ).astype(bf)  # [128, DCH, GC]
            # -> [128, 2(m), DCH, 128]
            m[nm] = np.ascontiguousarray(
                a.reshape(128, DCH, 2, 128).transpose(0, 2, 1, 3))
        m["wv"] = np.ascontiguousarray(
            Wv[cols, :].T.reshape(DCH, 128, GC).transpose(1, 0, 2)).astype(bf)
        m["wo"] = np.ascontiguousarray(
            Wo[:, cols].T.reshape(2, 128, D).transpose(1, 0, 2)).astype(bf)
        mb = np.where(mask[b, 0] != 0, 0.0, MASK_NEG).astype(np.float32)
        m["maskb"] = np.ascontiguousarray(mb.reshape(KC, 128).T)
        m["bq"] = np.ascontiguousarray(
            bq[cols].reshape(2, 128).T.astype(np.float32))
        m["bk"] = np.ascontiguousarray(
            bk[cols].reshape(2, 128).T.astype(np.float32))
        m["bv"] = np.ascontiguousarray(
            bv[cols].reshape(GH, 64).T.astype(np.float32))
        in_maps.append(m)
    return in_maps


def kernel(query, key, value, mask, Wq, bq, Wk, bk, Wv, bv, Wo, bo,
           _trace=False):
    query, key, value = (np.asarray(a, np.float32) for a in (query, key, value))
    mask = np.asarray(mask)
    with_bv = bool(np.any(np.asarray(bv)))
    nc = _get_program(with_bv)
    in_maps = make_in_maps(query, key, value, mask, Wq, bq, Wk, bk, Wv, bv,
                           Wo, bo)
    res = run_bass_kernel_spmd(nc, in_maps, list(range(NCORES)), trace=_trace)
    out = np.zeros((B, T, D), np.float32)
    for c in range(NCORES):
        out[c // 4] += np.asarray(res.results[c]["out"], np.float32).T
    out += np.asarray(bo, np.float32)[None, None, :]
    if _trace:
        kernel.last_exec_time_ns = res.exec_time_ns
        kernel.last_results = res
    return out
